# revision 25
# baseline (speedup 1.0000x reference)
"""AWQ quantized linear (4096 -> 11008) on 8 trn2 NeuronCores.

Column-parallel sharding: each core owns OUT/8 = 1376 output features.
Host side does only sharding + index-permutation (no arithmetic).

v5: fp8 DoubleRow main matmul with centered weights.
  Algebra: W = (q*s + o)*inv, with per-(o,g) group scale s, offset o and
  per-channel inv. Split  W = s*(q-7.5)*inv + (o + 7.5*s)*inv.
    - main term: u8 @ Wc8 where u8 = fp8e4(x) and
      Wc8 = fp8e4(s*(q-7.5)*inv*2^WS), run in DoubleRow mode (2 fp8
      channels per PE cell -> ~1.77x bf16 throughput). inv rides the
      W side (folded in the prep copy-out) so the x->fp8 cast is a
      bulk immediate op.
    - mean term (81% of weight energy, must stay precise): rank-1 per
      group: sum_g M[o,g] * V[t,g], V = group sums of x*inv. V^T is
      built per 512-token block on the PE via accumulating matmuls
      with inv-diagonal stationary tiles; the V@M^T term is 3 small
      K=33 matmuls per token tile (row NK carries the bias).
    - first KD 256-channel chunks ride the fp8 path; the remaining
      channels use plain fp16 weights (offset+inv folded) to keep
      total rel err ~1.87e-2 < 2e-2.
  Weight prep dequantizes in o-orientation ([P,1] scale/offset APs),
  PE-transposes to i-orientation, and the PSUM copy-out folds inv and
  the fp8/fp16 cast. Block-0 matmuls are WOVEN between prep chunks in
  emission order so the PE FIFO never starves while prep streams.
  PSUM runs at 2^WS; the output stage is one scalar activation copy
  with scale 2^-WS per slice.
"""

import sys

for _p in ("/opt/trn_rl_repo", "/opt/pypackages"):
    if _p not in sys.path:
        sys.path.append(_p)

import numpy as np

import concourse.bass as bass
import concourse.mybir as mybir
import concourse.tile as tile
from concourse import bacc
from concourse.bass_utils import run_bass_kernel_spmd
from concourse.masks import make_identity

IN = 4096
OUT = 11008
N_CORES = 8
OUT_SH = OUT // N_CORES  # 1376
T = 8192
NK = IN // 128  # 32 k-chunks of 128
P = 128
TB = 512       # token block for x staging

KD = 14        # fp8 DoubleRow chunks of 256 channels (2*KD 128-chunks)
WS = 10        # Wc8 = s*(q-7.5)*inv * 2^WS
SB = WS        # output descale 2^-SB

dt = mybir.dt
Alu = mybir.AluOpType
Act = mybir.ActivationFunctionType
PM = mybir.MatmulPerfMode


def build(n_t_tiles=T // P, out_sh=OUT_SH, kd=KD):
    n_o_tiles = (out_sh + P - 1) // P          # 11
    nc8 = 2 * kd                               # 128-chunks on the fp8 path
    nf = NK - nc8                              # 128-chunks on the fp16 path
    nsl = []
    n0 = 0
    while n0 < out_sh:
        nsz = min(512, out_sh - n0)
        nsl.append((n0, nsz))
        n0 += nsz

    n_tok = n_t_tiles * P
    tb = min(TB, n_tok)
    n_blocks = (n_tok + tb - 1) // tb
    tiles_per_block = tb // P

    nc = bacc.Bacc("TRN2", target_bir_lowering=False, debug=False,
                   num_devices=N_CORES)
    xt = nc.dram_tensor("xt", [IN, n_tok], dt.float32,
                        kind="ExternalInput").ap()
    pk = nc.dram_tensor("pk", [NK, P, n_o_tiles * 64], dt.int32,
                        kind="ExternalInput").ap()
    sc = nc.dram_tensor("sc", [P, n_o_tiles, NK], dt.float32,
                        kind="ExternalInput").ap()
    of = nc.dram_tensor("of", [P, n_o_tiles, NK], dt.float32,
                        kind="ExternalInput").ap()
    sct = nc.dram_tensor("sct", [NK, out_sh], dt.float32,
                         kind="ExternalInput").ap()
    oft = nc.dram_tensor("oft", [NK, out_sh], dt.float32,
                         kind="ExternalInput").ap()
    inv = nc.dram_tensor("inv", [P, NK], dt.float32,
                         kind="ExternalInput").ap()
    bias = nc.dram_tensor("bias", [out_sh], dt.float32,
                          kind="ExternalInput").ap()
    out = nc.dram_tensor("out", [n_tok, out_sh], dt.float32,
                         kind="ExternalOutput").ap()

    with tile.TileContext(nc) as tc:
        with (
            tc.tile_pool(name="const", bufs=1) as constp,
            tc.tile_pool(name="wtp", bufs=1) as wtp,
            tc.tile_pool(name="prep", bufs=4) as prep,
            tc.tile_pool(name="prepq", bufs=3) as prepq,
            tc.tile_pool(name="prepd", bufs=16) as prepd,
            tc.tile_pool(name="xtp", bufs=2) as xtp,
            tc.tile_pool(name="u8p", bufs=2) as u8p,
            tc.tile_pool(name="vtbp", bufs=2) as vtbp,
            tc.tile_pool(name="outp", bufs=4) as outp,
            tc.tile_pool(name="pmm", bufs=5, space="PSUM") as pmm,
            tc.tile_pool(name="ptp", bufs=2, space="PSUM") as ptp,
            tc.tile_pool(name="pvt", bufs=1, space="PSUM") as pvt,
        ):
            # first pk chunk first: heads the critical path
            pkt0 = prep.tile([P, n_o_tiles * 64], dt.int32, tag="pkt",
                             name="pkt0")
            nc.sync.dma_start(pkt0[:], pk[0])

            sc_all = constp.tile([P, n_o_tiles, NK], dt.float32)
            of_all = constp.tile([P, n_o_tiles, NK], dt.float32)
            nc.sync.dma_start(sc_all[:], sc[:])
            nc.sync.dma_start(of_all[:], of[:])

            # centered dequant constants for the fp8 path:
            #   wd = q*(s*2^WS) + (-7.5*s*2^WS)
            sc2 = constp.tile([P, n_o_tiles, NK], dt.float32)
            of2 = constp.tile([P, n_o_tiles, NK], dt.float32)
            nc.vector.tensor_scalar(sc2[:], sc_all[:], float(2 ** WS), None,
                                    op0=Alu.mult)
            nc.vector.tensor_scalar(of2[:], sc_all[:], float(-7.5 * 2 ** WS),
                                    None, op0=Alu.mult)

            ident16 = constp.tile([P, P], dt.float16)
            make_identity(nc, ident16[:])

            inv_sb = constp.tile([P, NK], dt.float32)
            nc.sync.dma_start(inv_sb[:], inv[:])
            invf_sb = constp.tile([P, NK], dt.float32)
            nc.vector.tensor_scalar(invf_sb[:], inv_sb[:], float(2 ** SB),
                                    None, op0=Alu.mult)
            # inv-diagonal stationary tiles for the V^T build
            inv_diag = constp.tile([P, NK, NK], dt.float16)
            nc.vector.memset(inv_diag[:], 0.0)
            for j in range(nc8):
                nc.vector.tensor_scalar(inv_diag[:, j, j:j + 1],
                                        inv_sb[:, j:j + 1], 1.0, None,
                                        op0=Alu.mult)

            # mean-term weights Mt[g, o] = (of[g,o] + 7.5*sc[g,o]) * 2^SB;
            # rows >= nc8 are dead (matching V rows are structurally 0).
            # Row NK carries the bias.
            sct_sb = constp.tile([NK, out_sh], dt.float32)
            oft_sb = constp.tile([NK, out_sh], dt.float32)
            nc.sync.dma_start(sct_sb[:], sct[:])
            nc.sync.dma_start(oft_sb[:], oft[:])
            mt16 = constp.tile([NK + 1, out_sh], dt.float16)
            nc.vector.tensor_scalar(sct_sb[:], sct_sb[:], 7.5, None,
                                    op0=Alu.mult)
            nc.vector.tensor_add(sct_sb[:], sct_sb[:], oft_sb[:])
            nc.vector.tensor_scalar(mt16[:NK], sct_sb[:], float(2 ** SB),
                                    None, op0=Alu.mult)
            nc.sync.dma_start(sct_sb[0:1, :], bias[None, :])
            nc.vector.tensor_scalar(mt16[NK:NK + 1], sct_sb[0:1, :],
                                    float(2 ** SB), None, op0=Alu.mult)

            wt8 = wtp.tile([P, nc8, out_sh], dt.float8e4)
            wt16 = wtp.tile([P, max(nf, 1), out_sh], dt.float16)

            # ---- weight prep chunk emitter (v1.5 deferred-copyout) ----
            pending = []

            def flush_pending():
                for ps_, c_, gs_, gn_ in pending:
                    o0 = gs_ * P
                    w = gn_ * P
                    h = (gn_ * P) // 2
                    if c_ < nc8:
                        dst = wt8[:, c_, :]
                        s_scale = inv_sb
                    else:
                        dst = wt16[:, c_ - nc8, :]
                        s_scale = invf_sb
                    hi = min(o0 + w, out_sh)
                    m = min(o0 + h, hi)
                    if m > o0:
                        nc.scalar.activation(dst[:, o0:m], ps_[:, :m - o0],
                                             Act.Copy,
                                             scale=s_scale[:, c_:c_ + 1])
                    if hi > m:
                        nc.vector.tensor_scalar(dst[:, m:hi],
                                                ps_[:, m - o0:hi - o0],
                                                s_scale[:, c_:c_ + 1], None,
                                                op0=Alu.mult)
                pending.clear()

            GRP = ((0, 4), (4, 4), (8, 3))

            def emit_prep_chunk(c):
                if c == 0:
                    pkt = pkt0
                else:
                    pkt = prep.tile([P, n_o_tiles * 64], dt.int32, tag="pkt",
                                    name=f"pkt{c}")
                    nc.sync.dma_start(pkt[:], pk[c])
                wq = prepq.tile([P, n_o_tiles * 64, 2], dt.int16, tag="wq",
                                name=f"wq{c}")
                pkt16 = pkt[:].bitcast(dt.int16).rearrange(
                    "p (a b) -> p a b", b=2)
                nc.vector.tensor_scalar(wq[:, :, 0], pkt16[:, :, 0], 15, None,
                                        op0=Alu.bitwise_and)
                nc.vector.tensor_scalar(wq[:, :, 1], pkt16[:, :, 0], 4, None,
                                        op0=Alu.logical_shift_right)
                s_ap = sc2 if c < nc8 else sc_all
                o_ap = of2 if c < nc8 else of_all
                wds = []
                for ot in range(n_o_tiles):
                    src = wq[:, ot * 64:(ot + 1) * 64, :].rearrange(
                        "p a b -> p (a b)")
                    wd = prepd.tile([P, P], dt.float16, tag="wd")
                    wds.append(wd)
                    if ot < 8:
                        nc.vector.tensor_scalar(
                            wd[:], src,
                            s_ap[:, ot, c:c + 1], o_ap[:, ot, c:c + 1],
                            op0=Alu.mult, op1=Alu.add)
                    else:
                        nc.scalar.activation(
                            wd[:], src, Act.Identity,
                            bias=o_ap[:, ot, c:c + 1],
                            scale=s_ap[:, ot, c:c + 1])
                flush_pending()
                for gi, (gs, gn) in enumerate(GRP):
                    ps = ptp.tile([P, 512], dt.float32, tag="tp")
                    for k in range(gn):
                        nc.tensor.matmul(ps[:, k * P:(k + 1) * P],
                                         lhsT=wds[gs + k][:],
                                         rhs=ident16[:], start=True,
                                         stop=True)
                    pending.append((ps, c, gs, gn))
                    if gi == 0:
                        flush_pending()

            # ---- block emitters ----
            def emit_block_dmas(b, order=None):
                xtb = xtp.tile([P, NK, tb], dt.float16, tag="xtb",
                               name=f"xtb{b}")
                for j in (order if order is not None else range(NK)):
                    nc.gpsimd.dma_start(
                        xtb[:, j, :],
                        xt[j * P:(j + 1) * P, b * tb:(b + 1) * tb])
                u8 = u8p.tile([P, nc8, tb], dt.float8e4, tag="u8",
                              name=f"u8_{b}")
                return xtb, u8

            def emit_u8_cast(xtb, u8, j0, j1, eng):
                if eng == 0:
                    nc.vector.tensor_scalar(u8[:, j0:j1, :], xtb[:, j0:j1, :],
                                            1.0, None, op0=Alu.mult)
                else:
                    nc.scalar.activation(u8[:, j0:j1, :], xtb[:, j0:j1, :],
                                         Act.Copy)

            def emit_vt_mm(b, xtb, vtb, j):
                nc.tensor.matmul(vtb[:, :], lhsT=inv_diag[:, j, :],
                                 rhs=xtb[:, j, :],
                                 start=(j == 0), stop=(j == nc8 - 1))

            def emit_vtsb(b, vtb):
                vtsb = vtbp.tile([NK + 1, tb], dt.float16, tag="vtsb",
                                 name=f"vtsb{b}")
                nc.vector.tensor_scalar(vtsb[:NK], vtb[:], 1.0, None,
                                        op0=Alu.mult)
                nc.vector.memset(vtsb[NK:NK + 1], 1.0)
                return vtsb

            def alloc_po(tt):
                return [pmm.tile([P, nsz], dt.float32, tag="po",
                                 name=f"po{tt}_{j}")
                        for j, (n0, nsz) in enumerate(nsl)]

            def emit_dr_pair(u8, po, it, c, slices=None, start=None):
                ts0 = it * P
                st = (c == 0) if start is None else start
                for j, (n0, nsz) in enumerate(nsl):
                    if slices is not None and j not in slices:
                        continue
                    nc.tensor.matmul(
                        po[j][:, :nsz],
                        lhsT=u8[:, 2 * c:2 * c + 2, ts0:ts0 + P],
                        rhs=wt8[:, 2 * c:2 * c + 2, n0:n0 + nsz],
                        start=st, stop=False,
                        perf_mode=PM.DoubleRow)

            def emit_fp16_chunk(xtb, po, it, f, start=False, slices=None):
                ts0 = it * P
                for j, (n0, nsz) in enumerate(nsl):
                    if slices is not None and j not in slices:
                        continue
                    nc.tensor.matmul(
                        po[j][:, :nsz],
                        lhsT=xtb[:, nc8 + f, ts0:ts0 + P],
                        rhs=wt16[:, f, n0:n0 + nsz],
                        start=start, stop=False)

            def emit_tile_tail(xtb, u8, vtsb, po, tt, it, dr_done=0,
                              skip_slices=(), fp16_done=False,
                              dr_start=True):
                ts0 = it * P
                t0 = tt * P
                sk = set(skip_slices)
                rest0 = tuple(j for j in range(len(nsl)) if j not in sk)
                for c in range(dr_done, kd):
                    emit_dr_pair(u8, po, it, c,
                                 slices=rest0 if sk else None,
                                 start=(c == 0 and dr_start))
                for j in skip_slices:
                    for c in range(kd):
                        emit_dr_pair(u8, po, it, c, slices=(j,))
                    for f in range(nf):
                        emit_fp16_chunk(xtb, po, it, f, slices=(j,))
                if not fp16_done:
                    sk = set(skip_slices)
                    rest = tuple(j for j in range(len(nsl)) if j not in sk)
                    for f in range(nf):
                        emit_fp16_chunk(xtb, po, it, f, slices=rest)
                for j, (n0, nsz) in enumerate(nsl):
                    nc.tensor.matmul(
                        po[j][:, :nsz],
                        lhsT=vtsb[:, ts0:ts0 + P],
                        rhs=mt16[:, n0:n0 + nsz],
                        start=False, stop=True)
                    osb = outp.tile([P, nsz], dt.float32, tag="osb",
                                    name=f"osb{tt}_{j}")
                    nc.scalar.activation(osb[:], po[j][:, :nsz], Act.Copy,
                                         scale=float(2.0 ** -SB))
                    nc.sync.dma_start(out[t0:t0 + P, n0:n0 + nsz], osb[:])

            # ---- woven prologue: prep all chunks (fp16 chunks first so
            # their matmuls weave early), with block-0 work interleaved
            # so the PE FIFO stays fed ----
            CORD = list(range(nc8, NK)) + list(range(nc8))
            xtb0, u80 = emit_block_dmas(0, order=CORD)
            vtb0 = pvt.tile([NK, tb], dt.float32, tag="vtb", name="vtb0")
            po0 = alloc_po(0)   # tile 0 (3 slices)
            po1 = alloc_po(1)   # tile 1; slice 2 deferred to the tail
            vtsb0 = None
            UPC = (1, 7, 13, 19)
            for ci, c in enumerate(CORD):
                emit_prep_chunk(c)
                if ci in UPC:
                    k = UPC.index(ci)
                    j0, j1 = (0, 7, 14, 21)[k], (7, 14, 21, nc8)[k]
                    emit_u8_cast(xtb0, u80, j0, j1, 1)
                if 1 <= ci <= nc8:
                    emit_vt_mm(0, xtb0, vtb0, ci - 1)
                    if ci == nc8:
                        vtsb0 = emit_vtsb(0, vtb0)
                if nf > 0 and 2 <= ci < 2 + nf:
                    f = ci - 2
                    emit_fp16_chunk(xtb0, po0, 0, f, start=(f == 0))
                    emit_fp16_chunk(xtb0, po1, 1, f, start=(f == 0),
                                    slices=(0, 1))
                if ci >= 7 and (ci - 7) % 2 == 0 and (ci - 7) // 2 < kd:
                    p = (ci - 7) // 2
                    emit_dr_pair(u80, po0, 0, p, start=(nf == 0 and p == 0))
                    emit_dr_pair(u80, po1, 1, p, start=(nf == 0 and p == 0),
                                 slices=(0, 1))
            flush_pending()
            woven = min((NK - 1 - 7) // 2 + 1, kd)  # DR pairs already woven
            emit_tile_tail(xtb0, u80, vtsb0, po0, 0, 0, dr_done=woven,
                           fp16_done=True, dr_start=False)
            emit_tile_tail(xtb0, u80, vtsb0, po1, 1, 1, dr_done=woven,
                           fp16_done=True, dr_start=False, skip_slices=(2,))
            for it in range(2, tiles_per_block):
                po = alloc_po(it)
                emit_tile_tail(xtb0, u80, vtsb0, po, it, it)

            # ---- steady-state blocks (next block's x DMA + u8 cast are
            # emitted one block early so they run during this block's
            # matmuls instead of stalling the next block's start) ----
            pending_blk = {}
            if n_blocks > 1:
                xtb1, u81 = emit_block_dmas(1)
                hh = nc8 // 2
                emit_u8_cast(xtb1, u81, 0, hh, 0)
                emit_u8_cast(xtb1, u81, hh, nc8, 0)
                pending_blk[1] = (xtb1, u81)
            for b in range(1, n_blocks):
                xtb, u8 = pending_blk.pop(b)
                vtb = pvt.tile([NK, tb], dt.float32, tag="vtb",
                               name=f"vtb{b}")
                for j in range(nc8):
                    emit_vt_mm(b, xtb, vtb, j)
                vtsb = emit_vtsb(b, vtb)
                if b + 1 < n_blocks:
                    xtbn, u8n = emit_block_dmas(b + 1)
                    hh = nc8 // 2
                    emit_u8_cast(xtbn, u8n, 0, hh, 0)
                    emit_u8_cast(xtbn, u8n, hh, nc8, 0)
                    pending_blk[b + 1] = (xtbn, u8n)
                for it in range(tiles_per_block):
                    tt = b * tiles_per_block + it
                    po = alloc_po(tt)
                    emit_tile_tail(xtb, u8, vtsb, po, tt, it)

    nc.compile()
    return nc


def make_in_maps(x, packed, scales, offsets, inv_scale, bias, out_sh=OUT_SH):
    n_o_tiles = (out_sh + P - 1) // P
    out_pad = n_o_tiles * P
    xf = np.asarray(x, dtype=np.float32).reshape(-1, IN)
    xth = np.ascontiguousarray(xf.T)
    pkm = np.asarray(packed, dtype=np.int32).reshape(OUT, IN // 2)
    scm = np.asarray(scales, dtype=np.float32).reshape(OUT, NK)
    ofm = np.asarray(offsets, dtype=np.float32).reshape(OUT, NK)
    invv = np.ascontiguousarray(
        np.asarray(inv_scale, dtype=np.float32).reshape(NK, P).T)
    bv = np.asarray(bias, dtype=np.float32)
    pad = out_pad - out_sh
    in_maps = []
    for k in range(N_CORES):
        sl = slice(k * out_sh, (k + 1) * out_sh)
        pk_k = np.pad(pkm[sl], ((0, pad), (0, 0)))
        pk_k = pk_k.reshape(n_o_tiles, P, NK, 64).transpose(2, 1, 0, 3)
        pk_k = np.ascontiguousarray(pk_k.reshape(NK, P, n_o_tiles * 64))
        sc_k = np.pad(scm[sl], ((0, pad), (0, 0)), constant_values=1.0)
        sc_k = np.ascontiguousarray(
            sc_k.reshape(n_o_tiles, P, NK).transpose(1, 0, 2))
        of_k = np.pad(ofm[sl], ((0, pad), (0, 0)))
        of_k = np.ascontiguousarray(
            of_k.reshape(n_o_tiles, P, NK).transpose(1, 0, 2))
        in_maps.append({
            "xt": xth,
            "pk": pk_k,
            "sc": sc_k,
            "of": of_k,
            "sct": np.ascontiguousarray(scm[sl].T),
            "oft": np.ascontiguousarray(ofm[sl].T),
            "inv": invv,
            "bias": np.ascontiguousarray(bv[sl]),
        })
    return in_maps


_CACHE = {}


def kernel(x, packed, scales, offsets, inv_scale, bias):
    if "nc" not in _CACHE:
        _CACHE["nc"] = build()
    nc = _CACHE["nc"]
    in_maps = make_in_maps(x, packed, scales, offsets, inv_scale, bias)
    res = run_bass_kernel_spmd(nc, in_maps, list(range(N_CORES)))
    cols = [res.results[k]["out"] for k in range(N_CORES)]
    full = np.concatenate(cols, axis=1)
    return np.ascontiguousarray(full.reshape(4, 2048, OUT).astype(np.float32))
